# revision 24
# baseline (speedup 1.0000x reference)
"""Trainium2 Bass kernel for nn_Attention_81071802679592.

Reference computation (B=4, L=2048, C=1024, H=16, D=64):
    qkv = x @ W_qkv.T + cat(q_bias, 0, v_bias)        -> q,k,v [B,H,L,D]
    q = q/||q|| * exp(min(scale_mul_log, ln 100));  k = k/||k||
    out = softmax(q @ k.T) @ v                        -> [B,L,C]
    y = out @ W_proj.T + b_proj

Sharding: 8 cores = (batch b = core//2) x (head-group g = core%2, 8 heads).
Each core computes a partial y^T for its batch restricted to its 8 heads;
the host sums the two head-group partials per batch and adds b_proj
(that is the tensor-parallel "all-reduce", done at unshard time).

Device layout choices (per core):
  - x is passed pre-transposed (xT [C, L]); QKV produces q^T,k^T in
    [feature, token] layout (feature partition-major, head-pair packed:
    rows 0-63 = head 2p, rows 64-127 = head 2p+1) and v in token-major
    [token, feature] layout (which is what the PV matmul wants).
  - scores are computed transposed: S^T[j,i] = sum_d k^T[d,j] q^T[d,i]
    with the two heads of a pair row-packed in the PE array (K=64 each).
  - softmax: exp on ScalarE (PSUM -> SBUF, fp16 out); the denominator is
    obtained by appending a ones-column to v (stationary [128,65]), so
    PV yields out^T (rows 0-63) and sum_j exp (row 64) in one pass.
  - proj consumes out^T directly; output is y^T (host transposes).
All matmul operands are fp16 (fp32 PSUM accumulate); norms/softmax
scaling in fp32.
"""

import os
import sys
from contextlib import ExitStack

import numpy as np

for _p in ("/opt/trn_rl_repo",):
    if _p not in sys.path:
        sys.path.insert(0, _p)

import concourse.bass as bass
import concourse.mybir as mybir
import concourse.tile as tile
from concourse import bacc
from concourse.alu_op_type import AluOpType
from concourse.bass import ts

F32 = mybir.dt.float32
F16 = mybir.dt.float16
EXP = mybir.ActivationFunctionType.Exp
SQRT = mybir.ActivationFunctionType.Sqrt

B, L, C, H, D = 4, 2048, 1024, 16, 64
NP = 4        # head pairs per core
CC = C // 128   # 8 contraction chunks
TB = L // 128   # 16 token tiles
KC = L // 128   # 16 key chunks
QH = 2          # q halves (1024 each)
MAX_SCALE_MUL = float(np.log(100.0))


def emit_kernel(ctx: ExitStack, tc: "tile.TileContext", io: dict):
    nc = tc.nc
    xTd, wqkTd, wvTd, qbd, vbd, shd, wpTd, yTd = (
        io["xT"], io["wqkT"], io["wvT"], io["qb"], io["vb"], io["sh"],
        io["wpT"], io["yT"],
    )

    # DRAM scratch for partition-broadcasts (row -> DRAM -> step-0 DMA back)
    scrd = nc.dram_tensor("scrd", [32, L], F32).ap()

    def bcast64(row, src_ap, dst_ap, n):
        nc.sync.dma_start(out=scrd[row:row + 1, 0:n], in_=src_ap)
        rep = bass.AP(tensor=scrd.tensor, offset=row * L,
                      ap=[[0, 64], [1, n]])
        nc.sync.dma_start(out=dst_ap, in_=rep)

    pin = ctx.enter_context(tc.tile_pool(name="pin", bufs=1))
    psA = ctx.enter_context(tc.tile_pool(name="psA", bufs=2, space="PSUM"))
    psB = ctx.enter_context(tc.tile_pool(name="psB", bufs=2, space="PSUM"))
    atp = ctx.enter_context(tc.tile_pool(name="atp", bufs=4))
    ctp = ctx.enter_context(tc.tile_pool(name="ctp", bufs=1))
    nrm = ctx.enter_context(tc.tile_pool(name="nrm", bufs=2))
    rbp = ctx.enter_context(tc.tile_pool(name="rbp", bufs=2))
    stg = ctx.enter_context(tc.tile_pool(name="stg", bufs=2))

    # ---- resident inputs -------------------------------------------------
    xT = pin.tile([128, CC, L], F16, tag="xT")          # x[b].T fp16
    for cc in range(CC):
        nc.sync.dma_start(out=xT[:, cc, :], in_=xTd[ts(cc, 128), :])
    wqk = pin.tile([128, CC, 1024], F16, tag="wqk")     # q|k feature cols
    for cc in range(CC):
        nc.sync.dma_start(out=wqk[:, cc, :], in_=wqkTd[ts(cc, 128), :])
    wv = pin.tile([128, CC, 512], F16, tag="wv")
    for cc in range(CC):
        nc.sync.dma_start(out=wv[:, cc, :], in_=wvTd[ts(cc, 128), :])
    wp = pin.tile([128, NP, 1024], F16, tag="wp")       # proj weights
    for fc in range(NP):
        nc.sync.dma_start(out=wp[:, fc, :], in_=wpTd[ts(fc, 128), :])
    qbr = pin.tile([1, 512], F16, tag="qbr")            # q_bias row (rank-1 MM)
    nc.sync.dma_start(out=qbr, in_=qbd)
    vbr = pin.tile([1, 512], F16, tag="vbr")            # v_bias row
    nc.sync.dma_start(out=vbr, in_=vbd)
    onesr = pin.tile([1, 512], F16, tag="onesr")
    nc.vector.memset(onesr, 1.0)
    sh = pin.tile([2, NP, 1], F32, tag="sh")            # per-head scale
    for pr in range(NP):
        nc.sync.dma_start(out=sh[:, pr, :], in_=shd[2 * pr:2 * pr + 2, :])

    # persistent intermediates
    qT = pin.tile([128, NP, L], F16, tag="qT")
    kT = pin.tile([128, NP, L], F16, tag="kT")
    vv = pin.tile([128, 2 * NP, KC, 64], F16, tag="vv")
    outT = pin.tile([128, NP, L], F16, tag="outT")

    e2 = pin.tile([128, 2], F16, tag="e2")              # per-head ones columns
    nc.vector.memset(e2, 0.0)
    nc.vector.memset(e2[0:64, 0:1], 1.0)
    nc.vector.memset(e2[64:128, 1:2], 1.0)
    e1 = pin.tile([128, 1], F16, tag="e1")              # all-ones (denominator)
    nc.vector.memset(e1, 1.0)

    # ---- QKV + norms, per pair ------------------------------------------
    for pr in range(NP):
        # q^T and k^T feature tiles ([128 feats, L]), via W^T stationary
        for which, dst in ((0, qT), (1, kT)):
            fbase = 512 * which + 128 * pr
            for half in range(2):
                ps = psA.tile([128, 1024], F32, tag="psA")
                for cc in range(CC):
                    for qb2 in range(2):
                        sl = ts(2 * half + qb2, 512)
                        nc.tensor.matmul(
                            ps[:, ts(qb2, 512)],
                            wqk[:, cc, fbase:fbase + 128],
                            xT[:, cc, sl],
                            start=(cc == 0),
                            stop=(which == 1 and cc == CC - 1))
                if which == 0:
                    # + q_bias as a rank-1 update: qb_col x ones_row
                    for qb2 in range(2):
                        nc.tensor.matmul(
                            ps[:, ts(qb2, 512)],
                            qbr[0:1, ts(pr, 128)], onesr,
                            start=False, stop=True)
                nc.vector.tensor_copy(
                    out=dst[:, pr, ts(half, 1024)], in_=ps)
        # v token-major tiles + bias, into v65 (ones col already set)
        for tb in range(TB):
            ps = psA.tile([128, 1024], F32, tag="psA")
            for cc in range(CC):
                nc.tensor.matmul(
                    ps[:, 0:128],
                    xT[:, cc, ts(tb, 128)],
                    wv[:, cc, ts(pr, 128)],
                    start=(cc == 0), stop=False)
            # + v_bias as a rank-1 update: ones_col x vb_row
            nc.tensor.matmul(
                ps[:, 0:128],
                onesr[0:1, 0:128], vbr[0:1, ts(pr, 128)],
                start=False, stop=True)
            for hh in range(2):
                nc.vector.tensor_copy(
                    out=vv[:, 2 * pr + hh, tb, :],
                    in_=ps[:, ts(hh, 64)])

        # L2 norms over d (partition dim) via ones-pair matmul
        for which, src in ((0, qT), (1, kT)):
            nsq = nrm.tile([2, L], F32, tag="nsq")
            for half in range(2):
                sq = atp.tile([128, 1024], F16, tag="at")
                nc.vector.tensor_mul(
                    out=sq, in0=src[:, pr, ts(half, 1024)],
                    in1=src[:, pr, ts(half, 1024)])
                psn = psA.tile([128, 1024], F32, tag="psA")
                for c2 in range(2):
                    nc.tensor.matmul(
                        psn[0:2, ts(c2, 512)], e2, sq[:, ts(c2, 512)],
                        start=True, stop=True)
                nc.vector.tensor_copy(out=nsq[:, ts(half, 1024)],
                                      in_=psn[0:2, :])
            # rinv = 1/sqrt(nsq)  (* scale for q)
            nc.scalar.activation(out=nsq, in_=nsq, func=SQRT)
            nc.vector.reciprocal(out=nsq, in_=nsq)
            if which == 0:
                nc.vector.tensor_scalar_mul(out=nsq, in0=nsq,
                                            scalar1=sh[:, pr, :])
            ct = ctp.tile([128, L], F32, tag="ct")
            base = 4 * pr + 2 * which
            bcast64(base + 0, nsq[0:1, :], ct[0:64, :], L)
            bcast64(base + 1, nsq[1:2, :], ct[64:128, :], L)
            nc.vector.tensor_tensor(out=src[:, pr, :], in0=src[:, pr, :],
                                    in1=ct, op=AluOpType.mult)

    # ---- attention, per pair / q-half -----------------------------------
    for pr in range(NP):
        for qh in range(QH):
            opv = psB.tile([128, 1024], F32, tag="psB")   # out^T both heads
            den = psB.tile([33, 1024], F32, tag="psB")    # denoms @ rows 0, 32
            for kc in range(KC):
                aTs = []
                for hh in range(2):
                    prt = slice(64 * hh, 64 * hh + 64)
                    psS = psA.tile([128, 1024], F32, tag="psA")
                    for qb2 in range(2):
                        nc.tensor.matmul(
                            psS[:, ts(qb2, 512)],
                            kT[prt, pr, ts(kc, 128)],
                            qT[prt, pr, 1024 * qh + 512 * qb2:
                               1024 * qh + 512 * qb2 + 512],
                            start=True, stop=True,
                            tile_position=(64 * hh, 0))
                    aT = atp.tile([128, 1024], F16, tag="at")
                    nc.scalar.activation(out=aT, in_=psS, func=EXP)
                    aTs.append(aT)
                st = (kc == 0)
                sp = (kc == KC - 1)
                for hh in range(2):
                    for qb2 in range(2):
                        # partition-disjoint groups share a bank: the sim's
                        # zero-region check is coarser than HW has_written
                        nc.tensor.matmul(
                            opv[64 * hh:64 * hh + 64, ts(qb2, 512)],
                            vv[:, 2 * pr + hh, kc, :],
                            aTs[hh][:, ts(qb2, 512)],
                            start=st, stop=sp,
                            tile_position=(0, 64 * hh),
                            skip_group_check=True)
                        nc.tensor.matmul(
                            den[32 * hh:32 * hh + 1, ts(qb2, 512)],
                            e1, aTs[hh][:, ts(qb2, 512)],
                            start=st, stop=sp,
                            tile_position=(0, 32 * hh),
                            skip_group_check=True)
            # normalize by denominator and store out^T (fp16)
            ctd = ctp.tile([128, L], F32, tag="ct")
            rb33 = rbp.tile([33, 1024], F32, tag="rb")
            nc.vector.reciprocal(out=rb33[0:1, :], in_=den[0:1, :])
            nc.vector.reciprocal(out=rb33[32:33, :], in_=den[32:33, :])
            base = 16 + 4 * pr + 2 * qh
            bcast64(base + 0, rb33[0:1, :], ctd[0:64, 0:1024], 1024)
            bcast64(base + 1, rb33[32:33, :], ctd[64:128, 0:1024], 1024)
            osl = slice(1024 * qh, 1024 * qh + 1024)
            nc.vector.tensor_tensor(
                out=outT[:, pr, osl], in0=opv,
                in1=ctd[:, 0:1024], op=AluOpType.mult)

    # ---- output projection (y^T = Wp_sub @ out_tok^T) --------------------
    for oc in range(8):
        for tb2 in range(2):
            ps = psA.tile([128, 1024], F32, tag="psA")
            for fc in range(NP):
                for qb2 in range(2):
                    sl = ts(2 * tb2 + qb2, 512)
                    nc.tensor.matmul(
                        ps[:, ts(qb2, 512)],
                        wp[:, fc, ts(oc, 128)],
                        outT[:, fc, sl],
                        start=(fc == 0), stop=(fc == NP - 1))
            st = stg.tile([128, 1024], F32, tag="stg")
            nc.vector.tensor_copy(out=st, in_=ps)
            nc.sync.dma_start(out=yTd[ts(oc, 128), ts(tb2, 1024)], in_=st)


def build_nc():
    nc = bacc.Bacc("TRN2", target_bir_lowering=False, debug=False,
                   enable_asserts=False)
    io = {
        "xT": nc.dram_tensor("xT", [C, L], F16, kind="ExternalInput").ap(),
        "wqkT": nc.dram_tensor("wqkT", [C, 1024], F16,
                               kind="ExternalInput").ap(),
        "wvT": nc.dram_tensor("wvT", [C, 512], F16,
                              kind="ExternalInput").ap(),
        "qb": nc.dram_tensor("qb", [1, 512], F16, kind="ExternalInput").ap(),
        "vb": nc.dram_tensor("vb", [1, 512], F16, kind="ExternalInput").ap(),
        "sh": nc.dram_tensor("sh", [8, 1], F32, kind="ExternalInput").ap(),
        "wpT": nc.dram_tensor("wpT", [512, 1024], F16,
                              kind="ExternalInput").ap(),
        "yT": nc.dram_tensor("yT", [C, L], F32, kind="ExternalOutput").ap(),
    }
    with tile.TileContext(nc) as tc:
        with ExitStack() as ctx:
            emit_kernel(ctx, tc, io)
    nc.compile()
    return nc


def prep_core_inputs(core, x, W_qkv, q_bias, v_bias, scale_mul_log, W_proj):
    b, g = divmod(core, 2)
    heads = np.arange(8 * g, 8 * g + 8)
    qf = (heads[:, None] * 64 + np.arange(64)[None, :]).reshape(-1)
    scale = np.exp(np.minimum(scale_mul_log.reshape(H), MAX_SCALE_MUL))
    return {
        "xT": np.ascontiguousarray(x[b].T).astype(np.float16),
        "wqkT": np.ascontiguousarray(
            np.concatenate([W_qkv[qf, :], W_qkv[1024 + qf, :]], 0).T
        ).astype(np.float16),
        "wvT": np.ascontiguousarray(W_qkv[2048 + qf, :].T).astype(np.float16),
        "qb": np.ascontiguousarray(q_bias[qf].reshape(1, 512)).astype(
            np.float16),
        "vb": np.ascontiguousarray(v_bias[qf].reshape(1, 512)).astype(
            np.float16),
        "sh": np.ascontiguousarray(scale[heads].reshape(8, 1)).astype(
            np.float32),
        "wpT": np.ascontiguousarray(W_proj[:, qf].T).astype(np.float16),
    }


_NC_CACHE = {}


def kernel(x, W_qkv, q_bias, v_bias, scale_mul_log, W_proj, b_proj,
           _trace=False):
    x = np.asarray(x, dtype=np.float32)
    W_qkv = np.asarray(W_qkv, dtype=np.float32)
    q_bias = np.asarray(q_bias, dtype=np.float32)
    v_bias = np.asarray(v_bias, dtype=np.float32)
    scale_mul_log = np.asarray(scale_mul_log, dtype=np.float32)
    W_proj = np.asarray(W_proj, dtype=np.float32)
    b_proj = np.asarray(b_proj, dtype=np.float32)

    from concourse.bass_utils import run_bass_kernel_spmd

    if "nc" not in _NC_CACHE:
        _NC_CACHE["nc"] = build_nc()
    nc = _NC_CACHE["nc"]

    in_maps = [
        prep_core_inputs(c, x, W_qkv, q_bias, v_bias, scale_mul_log, W_proj)
        for c in range(8)
    ]
    res = run_bass_kernel_spmd(nc, in_maps, core_ids=list(range(8)),
                               trace=_trace)
    if _trace:
        kernel.last_results = res

    y = np.empty((B, L, C), dtype=np.float32)
    for b in range(B):
        y[b] = (res.results[2 * b]["yT"].T + res.results[2 * b + 1]["yT"].T
                + b_proj)
    return y


if __name__ == "__main__":
    print("building program...")
    nc = build_nc()
    print("built ok")


# revision 36
# speedup vs baseline: 1.1745x; 1.1745x over previous
"""Trainium2 Bass kernel for nn_Attention_81071802679592.

Reference computation (B=4, L=2048, C=1024, H=16, D=64):
    qkv = x @ W_qkv.T + cat(q_bias, 0, v_bias)        -> q,k,v [B,H,L,D]
    q = q/||q|| * exp(min(scale_mul_log, ln 100));  k = k/||k||
    out = softmax(q @ k.T) @ v                        -> [B,L,C]
    y = out @ W_proj.T + b_proj

Sharding: 8 cores = (batch b = core//2) x (head-group g = core%2, 8 heads).
Each core computes a partial y^T for its batch restricted to its 8 heads;
the host sums the two head-group partials per batch and adds b_proj
(that is the tensor-parallel "all-reduce", done at unshard time).

Device layout choices (per core):
  - x is passed pre-transposed (xT [C, L]); QKV produces q^T,k^T in
    [feature, token] layout (feature partition-major, head-pair packed:
    rows 0-63 = head 2p, rows 64-127 = head 2p+1) and v in token-major
    [token, feature] layout (which is what the PV matmul wants).
  - scores are computed transposed: S^T[j,i] = sum_d k^T[d,j] q^T[d,i]
    with the two heads of a pair row-packed in the PE array (K=64 each).
  - softmax: exp on ScalarE (PSUM -> SBUF, fp16 out); the denominator is
    obtained by appending a ones-column to v (stationary [128,65]), so
    PV yields out^T (rows 0-63) and sum_j exp (row 64) in one pass.
  - proj consumes out^T directly; output is y^T (host transposes).
All matmul operands are fp16 (fp32 PSUM accumulate); norms/softmax
scaling in fp32.
"""

import os
import sys
from contextlib import ExitStack

import numpy as np

for _p in ("/opt/trn_rl_repo",):
    if _p not in sys.path:
        sys.path.insert(0, _p)

import concourse.bass as bass
import concourse.mybir as mybir
import concourse.tile as tile
from concourse import bacc
from concourse.alu_op_type import AluOpType
from concourse.bass import ts

F32 = mybir.dt.float32
F16 = mybir.dt.float16
EXP = mybir.ActivationFunctionType.Exp
SQRT = mybir.ActivationFunctionType.Sqrt

B, L, C, H, D = 4, 2048, 1024, 16, 64
NP = 4        # head pairs per core
CC = C // 128   # 8 contraction chunks
TB = L // 128   # 16 token tiles
KC = L // 128   # 16 key chunks
QH = 2          # q halves (1024 each)
MAX_SCALE_MUL = float(np.log(100.0))


def emit_kernel(ctx: ExitStack, tc: "tile.TileContext", io: dict):
    nc = tc.nc
    xTd, wqkTd, wvTd, qbd, vbd, shd, wpTd, yTd = (
        io["xT"], io["wqkT"], io["wvT"], io["qb"], io["vb"], io["sh"],
        io["wpT"], io["yT"],
    )

    # DRAM scratch for partition-broadcasts (row -> DRAM -> step-0 DMA back)
    scrd = nc.dram_tensor("scrd", [32, L], F32).ap()

    def bcast2(row, src2_ap, dst_ap, n):
        """src2 [2, n] -> dst [128, n]: row i of src replicated to 64
        partitions (via a DRAM bounce; DMA replicates with step-0 APs)."""
        nc.sync.dma_start(out=scrd[row:row + 2, 0:n], in_=src2_ap)
        for i in range(2):
            rep = bass.AP(tensor=scrd.tensor, offset=(row + i) * L,
                          ap=[[0, 64], [1, n]])
            nc.sync.dma_start(out=dst_ap[64 * i:64 * i + 64, 0:n], in_=rep)

    pin = ctx.enter_context(tc.tile_pool(name="pin", bufs=1))
    psA = ctx.enter_context(tc.tile_pool(name="psA", bufs=2, space="PSUM"))
    psB = ctx.enter_context(tc.tile_pool(name="psB", bufs=2, space="PSUM"))
    atp = ctx.enter_context(tc.tile_pool(name="atp", bufs=4))
    ctp = ctx.enter_context(tc.tile_pool(name="ctp", bufs=1))
    nrm = ctx.enter_context(tc.tile_pool(name="nrm", bufs=2))
    rbp = ctx.enter_context(tc.tile_pool(name="rbp", bufs=2))
    stg = ctx.enter_context(tc.tile_pool(name="stg", bufs=2))

    # ---- resident inputs -------------------------------------------------
    xT = pin.tile([128, CC, L], F16, tag="xT")          # x[b].T fp16
    for cc in range(CC):
        nc.sync.dma_start(out=xT[:, cc, :], in_=xTd[ts(cc, 128), :])
    wqk = pin.tile([128, CC, 1024], F16, tag="wqk")     # q|k feature cols
    for cc in range(CC):
        nc.sync.dma_start(out=wqk[:, cc, :], in_=wqkTd[ts(cc, 128), :])
    wv = pin.tile([128, CC, 512], F16, tag="wv")
    for cc in range(CC):
        nc.sync.dma_start(out=wv[:, cc, :], in_=wvTd[ts(cc, 128), :])
    wp = pin.tile([128, NP, 1024], F16, tag="wp")       # proj weights
    for fc in range(NP):
        nc.sync.dma_start(out=wp[:, fc, :], in_=wpTd[ts(fc, 128), :])
    qbr = pin.tile([1, 512], F16, tag="qbr")            # q_bias row (rank-1 MM)
    nc.sync.dma_start(out=qbr, in_=qbd)
    vbr = pin.tile([1, 512], F16, tag="vbr")            # v_bias row
    nc.sync.dma_start(out=vbr, in_=vbd)
    onesr = pin.tile([1, 512], F16, tag="onesr")
    nc.vector.memset(onesr, 1.0)
    sh = pin.tile([2, NP, 1], F32, tag="sh")            # per-head scale^-2
    for pr in range(NP):
        nc.sync.dma_start(out=sh[:, pr, :], in_=shd[2 * pr:2 * pr + 2, :])

    # persistent intermediates
    qT = pin.tile([128, NP, L], F16, tag="qT")
    kT = pin.tile([128, NP, L], F16, tag="kT")
    vv = pin.tile([128, 2 * NP, KC, 64], F16, tag="vv")
    outT = pin.tile([128, NP, L], F16, tag="outT")

    e2 = pin.tile([128, 2], F16, tag="e2")              # per-head ones columns
    nc.vector.memset(e2, 0.0)
    nc.vector.memset(e2[0:64, 0:1], 1.0)
    nc.vector.memset(e2[64:128, 1:2], 1.0)
    e1 = pin.tile([128, 1], F16, tag="e1")              # all-ones (denominator)
    nc.vector.memset(e1, 1.0)

    # ---- QKV + norms, per pair ------------------------------------------
    for pr in range(NP):
        # q^T and k^T feature tiles ([128 feats, L]), via W^T stationary
        for which, dst in ((0, qT), (1, kT)):
            fbase = 512 * which + 128 * pr
            for half in range(2):
                ps = psA.tile([128, 1024], F32, tag="psA")
                for cc in range(CC):
                    for qb2 in range(2):
                        sl = ts(2 * half + qb2, 512)
                        nc.tensor.matmul(
                            ps[:, ts(qb2, 512)],
                            wqk[:, cc, fbase:fbase + 128],
                            xT[:, cc, sl],
                            start=(cc == 0),
                            stop=(which == 1 and cc == CC - 1))
                if which == 0:
                    # + q_bias as a rank-1 update: qb_col x ones_row
                    for qb2 in range(2):
                        nc.tensor.matmul(
                            ps[:, ts(qb2, 512)],
                            qbr[0:1, ts(pr, 128)], onesr,
                            start=False, stop=True)
                nc.vector.tensor_copy(
                    out=dst[:, pr, ts(half, 1024)], in_=ps)
        # L2 norms over d (partition dim) via ones-pair matmul
        for which, src in ((0, qT), (1, kT)):
            nsq = nrm.tile([2, L], F32, tag="nsq")
            for half in range(2):
                sq = atp.tile([128, 1024], F16, tag="at")
                nc.vector.tensor_mul(
                    out=sq, in0=src[:, pr, ts(half, 1024)],
                    in1=src[:, pr, ts(half, 1024)])
                psn = psA.tile([128, 1024], F32, tag="psA")
                for c2 in range(2):
                    nc.tensor.matmul(
                        psn[0:2, ts(c2, 512)], e2, sq[:, ts(c2, 512)],
                        start=True, stop=True)
                nc.vector.tensor_copy(out=nsq[:, ts(half, 1024)],
                                      in_=psn[0:2, :])
            # rinv = 1/sqrt(nsq * scale^-2) = scale/||.||  (scale=1 for k)
            if which == 0:
                nc.scalar.activation(out=nsq, in_=nsq, func=SQRT,
                                     scale=sh[:, pr, :])
            else:
                nc.scalar.activation(out=nsq, in_=nsq, func=SQRT)
            nc.vector.reciprocal_approx_fast(out=nsq, in_=nsq)
            ct = ctp.tile([128, L], F32, tag="ct")
            bcast2(4 * pr + 2 * which, nsq, ct, L)
            nc.vector.tensor_tensor(out=src[:, pr, :], in0=src[:, pr, :],
                                    in1=ct, op=AluOpType.mult)

        if pr == 0:
            # v token-major for ALL pairs (N=512 matmuls) + bias, into vv
            for tb in range(TB):
                ps = psA.tile([128, 1024], F32, tag="psA")
                for cc in range(CC):
                    nc.tensor.matmul(
                        ps[:, 0:512],
                        xT[:, cc, ts(tb, 128)],
                        wv[:, cc, :],
                        start=(cc == 0), stop=False)
                nc.tensor.matmul(
                    ps[:, 0:512],
                    onesr[0:1, 0:128], vbr,
                    start=False, stop=True)
                for p2 in range(NP):
                    nc.vector.tensor_copy(
                        out=vv[:, 2 * p2:2 * p2 + 2, tb, :],
                        in_=ps[:, ts(p2, 128)].rearrange(
                            "p (h d) -> p h d", h=2))

    # ---- attention, per pair / q-half -----------------------------------
    for pr in range(NP):
        for qh in range(QH):
            opv = psB.tile([128, 1024], F32, tag="psB")   # out^T both heads
            den = psB.tile([33, 1024], F32, tag="psB")    # denoms @ rows 0, 32
            for kc in range(KC):
                aTs = []
                for hh in range(2):
                    prt = slice(64 * hh, 64 * hh + 64)
                    psS = psA.tile([128, 1024], F32, tag="psA")
                    for qb2 in range(2):
                        nc.tensor.matmul(
                            psS[:, ts(qb2, 512)],
                            kT[prt, pr, ts(kc, 128)],
                            qT[prt, pr, 1024 * qh + 512 * qb2:
                               1024 * qh + 512 * qb2 + 512],
                            start=True, stop=True,
                            tile_position=(64 * hh, 0))
                    aT = atp.tile([128, 1024], F16, tag="at")
                    nc.scalar.activation(out=aT, in_=psS, func=EXP)
                    aTs.append(aT)
                st = (kc == 0)
                sp = (kc == KC - 1)
                for hh in range(2):
                    for qb2 in range(2):
                        # partition-disjoint groups share a bank: the sim's
                        # zero-region check is coarser than HW has_written
                        nc.tensor.matmul(
                            opv[64 * hh:64 * hh + 64, ts(qb2, 512)],
                            vv[:, 2 * pr + hh, kc, :],
                            aTs[hh][:, ts(qb2, 512)],
                            start=st, stop=sp,
                            tile_position=(0, 64 * hh),
                            skip_group_check=True)
                        nc.tensor.matmul(
                            den[32 * hh:32 * hh + 1, ts(qb2, 512)],
                            e1, aTs[hh][:, ts(qb2, 512)],
                            start=st, stop=sp,
                            tile_position=(0, 32 * hh),
                            skip_group_check=True)
            # drain psum promptly (frees slots for the next block): out^T to
            # fp16, raw denominator rows to SBUF; then broadcast the raw
            # denominators and take the fast reciprocal on the full base-0
            # tile (reciprocal_approx_fast is broken at partition base != 0)
            ov16 = atp.tile([128, 1024], F16, tag="at")
            nc.vector.tensor_copy(out=ov16, in_=opv)
            rb33 = rbp.tile([33, 1024], F32, tag="rb")
            nc.vector.tensor_copy(out=rb33[0:1, :], in_=den[0:1, :])
            nc.vector.tensor_copy(out=rb33[32:33, :], in_=den[32:33, :])
            ctd = ctp.tile([128, L], F32, tag="ct")
            bcast2(16 + 4 * pr + 2 * qh, rb33[0:33:32, :], ctd[:, 0:1024],
                   1024)
            nc.vector.reciprocal_approx_fast(out=ctd[:, 0:1024],
                                             in_=ctd[:, 0:1024])
            osl = slice(1024 * qh, 1024 * qh + 1024)
            nc.vector.tensor_tensor(
                out=outT[:, pr, osl], in0=ov16,
                in1=ctd[:, 0:1024], op=AluOpType.mult)

    # ---- output projection (y^T = Wp_sub @ out_tok^T) --------------------
    for oc in range(8):
        for tb2 in range(2):
            ps = psA.tile([128, 1024], F32, tag="psA")
            for fc in range(NP):
                for qb2 in range(2):
                    sl = ts(2 * tb2 + qb2, 512)
                    nc.tensor.matmul(
                        ps[:, ts(qb2, 512)],
                        wp[:, fc, ts(oc, 128)],
                        outT[:, fc, sl],
                        start=(fc == 0), stop=(fc == NP - 1))
            st = stg.tile([128, 1024], F32, tag="stg")
            nc.vector.tensor_copy(out=st, in_=ps)
            nc.sync.dma_start(out=yTd[ts(oc, 128), ts(tb2, 1024)], in_=st)


def build_nc():
    nc = bacc.Bacc("TRN2", target_bir_lowering=False, debug=False,
                   enable_asserts=False)
    io = {
        "xT": nc.dram_tensor("xT", [C, L], F16, kind="ExternalInput").ap(),
        "wqkT": nc.dram_tensor("wqkT", [C, 1024], F16,
                               kind="ExternalInput").ap(),
        "wvT": nc.dram_tensor("wvT", [C, 512], F16,
                              kind="ExternalInput").ap(),
        "qb": nc.dram_tensor("qb", [1, 512], F16, kind="ExternalInput").ap(),
        "vb": nc.dram_tensor("vb", [1, 512], F16, kind="ExternalInput").ap(),
        "sh": nc.dram_tensor("sh", [8, 1], F32, kind="ExternalInput").ap(),
        "wpT": nc.dram_tensor("wpT", [512, 1024], F16,
                              kind="ExternalInput").ap(),
        "yT": nc.dram_tensor("yT", [C, L], F32, kind="ExternalOutput").ap(),
    }
    with tile.TileContext(nc) as tc:
        with ExitStack() as ctx:
            emit_kernel(ctx, tc, io)
    nc.compile()
    return nc


def prep_core_inputs(core, x, W_qkv, q_bias, v_bias, scale_mul_log, W_proj):
    b, g = divmod(core, 2)
    heads = np.arange(8 * g, 8 * g + 8)
    qf = (heads[:, None] * 64 + np.arange(64)[None, :]).reshape(-1)
    scale = np.exp(np.minimum(scale_mul_log.reshape(H), MAX_SCALE_MUL))
    return {
        "xT": np.ascontiguousarray(x[b].T).astype(np.float16),
        "wqkT": np.ascontiguousarray(
            np.concatenate([W_qkv[qf, :], W_qkv[1024 + qf, :]], 0).T
        ).astype(np.float16),
        "wvT": np.ascontiguousarray(W_qkv[2048 + qf, :].T).astype(np.float16),
        "qb": np.ascontiguousarray(q_bias[qf].reshape(1, 512)).astype(
            np.float16),
        "vb": np.ascontiguousarray(v_bias[qf].reshape(1, 512)).astype(
            np.float16),
        "sh": np.ascontiguousarray(
            (scale[heads] ** -2.0).reshape(8, 1)).astype(np.float32),
        "wpT": np.ascontiguousarray(W_proj[:, qf].T).astype(np.float16),
    }


_NC_CACHE = {}


def kernel(x, W_qkv, q_bias, v_bias, scale_mul_log, W_proj, b_proj,
           _trace=False):
    x = np.asarray(x, dtype=np.float32)
    W_qkv = np.asarray(W_qkv, dtype=np.float32)
    q_bias = np.asarray(q_bias, dtype=np.float32)
    v_bias = np.asarray(v_bias, dtype=np.float32)
    scale_mul_log = np.asarray(scale_mul_log, dtype=np.float32)
    W_proj = np.asarray(W_proj, dtype=np.float32)
    b_proj = np.asarray(b_proj, dtype=np.float32)

    from concourse.bass_utils import run_bass_kernel_spmd

    if "nc" not in _NC_CACHE:
        _NC_CACHE["nc"] = build_nc()
    nc = _NC_CACHE["nc"]

    in_maps = [
        prep_core_inputs(c, x, W_qkv, q_bias, v_bias, scale_mul_log, W_proj)
        for c in range(8)
    ]
    res = run_bass_kernel_spmd(nc, in_maps, core_ids=list(range(8)),
                               trace=_trace)
    if _trace:
        kernel.last_results = res

    y = np.empty((B, L, C), dtype=np.float32)
    for b in range(B):
        y[b] = (res.results[2 * b]["yT"].T + res.results[2 * b + 1]["yT"].T
                + b_proj)
    return y


if __name__ == "__main__":
    print("building program...")
    nc = build_nc()
    print("built ok")
